# revision 2
# baseline (speedup 1.0000x reference)
"""Trainium2 kernel for a fuzzy-logic ConjunctionLayer forward pass.

Computes  out = 1[ (1 - x) @ 1[W > 0.5] <= 0 ]  for
x: [8192, 4096] f32, W: [4096, 2048] f32 -> out: [8192, 2048] f32.

Sharding: data-parallel over the batch dim across 8 NeuronCores
(x shard [1024, 4096] per core, W replicated), outputs concatenated.

Math: with x in [0, 1], every term (1-x)*Wb is >= 0, so
  res[m,n] <= 0  <=>  res[m,n] == 0  <=>  no k has (1-x[m,k] > 0 AND W[k,n] > .5).
Because all summands are nonnegative, the predicate `res > 0` is exact
under ANY rounding of the summands or the f32 PSUM accumulation: the
sum is zero iff every term is zero. So both operands ship as lossy fp8
as long as zeros/positives are preserved:
  xe = rtp8(1 - x)   (round-toward-+inf; 0 iff x >= 1; subnormals
                      promoted to 2^-6 so the PE never sees them)
  W  = rtp8(W)       (0.5 representable => rtp(W) > 0.5 <=> W > 0.5)
On device:
  wb = threshold(W)  split across two engines, both exact:
         DVE:  is_gt(W, 0.5)        -> {0, 1}
         ACT:  relu(W - 0.5)        -> {0} u [0.0625, 0.5]
  acc = xe^T @ wb    (fp8 DoubleRow matmul, f32 PSUM)
  out = 1[acc <= 0]  (DVE is_le, fp8 {0,1}), widened to f32 on host.
The device performs all thresholding of W, all matmuls, and the output
classification; x ships as the actual fuzzy values (1-x) like the
reference consumes.

Schedule (the PE-bound part): the 512 matmuls are grouped so one
stationary LDWEIGHTS (the x side, [128, 2, 128]) feeds FOUR consecutive
matmuls - the four 512-wide n-blocks - accumulating into a 4-bank PSUM
tile [128, 2048] f32. Two m-chunks (one "m-pair") are in flight at a
time = 8 PSUM banks. In fp8 DoubleRow mode the PE cannot double-buffer
weight loads (both planes hold the packed rows), so LDWEIGHTS serializes
with the matmul stream; a post-scheduling IR pass removes the redundant
LDWEIGHTS the legalizer emits for matmuls 2-4 of each group (the PE
keeps its weights between matmuls), merging their semaphore waits into
the adjacent matmul. This cuts PE occupancy from (512 LDW + 512 MM) to
(128 LDW + 512 MM).

DMA: 28 large transfers (x: 4 x 1MB pair-major slabs on the Scalar
HWDGE ring, W: 16 x 512KB kk-major on the Sync ring, out: 8 x 256KB
row-blocks on the Scalar ring). W binarization runs in place as W
slabs land; m-pair 0 rides the W stream, later pairs run PE-bound from
resident tiles.
"""

import os

import numpy as np

import concourse.bass as bass
import concourse.mybir as mybir
import concourse.tile as tile
from concourse import bacc
from concourse.bass_utils import run_bass_kernel_spmd

BATCH, IN_DIM, N_RULES = 8192, 4096, 2048
N_CORES = 8
M_LOCAL = BATCH // N_CORES  # 1024 batch rows per core

P = 128                     # SBUF partitions / matmul tile edge
KP = IN_DIM // (2 * P)      # 16 k-pairs (DoubleRow consumes 2 k-tiles)
NB = 4                      # n-blocks of 512 (one f32 PSUM bank each)
NB_W = N_RULES // NB        # 512
NPAIR = 4                   # m-pair phases (2 m-chunks each)
MI = 2                      # m-chunks per pair

F32 = mybir.dt.float32
FP8 = mybir.dt.float8e4
ALU = mybir.AluOpType
DR = mybir.MatmulPerfMode.DoubleRow
AF = mybir.ActivationFunctionType

# W-binarize engine split point (columns of the [128, 4096] W slab):
# DVE takes [0, DVE_COLS), ACT relu takes the rest.
DVE_COLS = 2560

DEDUP_LDW = os.environ.get("KBENCH_NO_DEDUP", "") != "1"


def _body(tc: tile.TileContext, out: bass.AP, xp: bass.AP, wp: bass.AP):
    nc = tc.nc
    with (
        tc.tile_pool(name="sb", bufs=1) as sb,
        tc.tile_pool(name="ps", bufs=1, space="PSUM") as ps,
    ):
        # per-partition -0.5 bias for the ACT-engine relu threshold
        bias = sb.tile([P, 1], F32, tag="bias", bufs=1, name="bias")
        nc.gpsimd.memset(bias[:], -0.5)

        # Resident operand tiles: 4 x-pair slabs (1MB each) and 16 W
        # slabs (512KB each, binarized in place).
        sx = [sb.tile([P, KP * 2 * MI * P], FP8, tag=f"sx{p}", bufs=1,
                      name=f"sx{p}") for p in range(NPAIR)]
        wb = [sb.tile([P, NB * 2 * NB_W], FP8, tag=f"wb{k}", bufs=1,
                      name=f"wb{k}") for k in range(KP)]

        def load_x(pair):
            nc.scalar.dma_start(sx[pair][:], xp[pair])

        def load_w(kk):
            a = wb[kk][:]
            nc.sync.dma_start(a, wp[kk])
            # in-place thresholding, split across engines (disjoint cols)
            nc.vector.tensor_scalar(a[:, :DVE_COLS], a[:, :DVE_COLS],
                                    0.5, None, ALU.is_gt)
            nc.scalar.activation(a[:, DVE_COLS:], a[:, DVE_COLS:],
                                 AF.Relu, bias=bias[:], scale=1.0)

        # Upfront load stream: x pair-slabs staggered through the
        # kk-major W stream (x on the Scalar ring, W on the Sync ring).
        load_x(0)
        for kk in range(KP):
            load_w(kk)
            if kk == 4:
                load_x(1)
            elif kk == 9:
                load_x(2)
            elif kk == 13:
                load_x(3)

        def lhsT_ap(pair, kk, mi):
            # [128, 2, 128] stationary: x-pair slab cols
            # kk*512 + j*256 + mi*128 + m
            sl = sx[pair][:][:, kk * 512:(kk + 1) * 512]
            sl = sl.rearrange("p (two mm) -> p two mm", two=2)
            return sl[:, :, mi * P:(mi + 1) * P]

        def rhs_ap(kk, nb):
            # [128, 2, 512] moving: W slab cols nb*1024 + j*512 + n
            sl = wb[kk][:][:, nb * 1024:(nb + 1) * 1024]
            return sl.rearrange("p (two n) -> p two n", two=2)

        for pair in range(NPAIR):
            accs = [ps.tile([P, N_RULES], F32, tag=f"acc{mi}", bufs=1,
                            name=f"acc{pair}_{mi}") for mi in range(MI)]
            for kk in range(KP):
                for mi in range(MI):
                    lhsT = lhsT_ap(pair, kk, mi)
                    for nb in range(NB):
                        nc.tensor.matmul(
                            accs[mi][:][:, nb * NB_W:(nb + 1) * NB_W],
                            lhsT,
                            rhs_ap(kk, nb),
                            start=(kk == 0),
                            stop=(kk == KP - 1),
                            perf_mode=DR,
                        )
            for mi in range(MI):
                mch = pair * MI + mi
                o = sb.tile([P, N_RULES], FP8, tag="o", bufs=4,
                            name=f"o{mch}")
                nc.vector.tensor_scalar(o[:], accs[mi][:], 0.0, None,
                                        ALU.is_le)
                nc.scalar.dma_start(out[mch * P:(mch + 1) * P, :], o[:])


def _merge_sync(dst, extra):
    """Merge `extra` (a SyncInfo or None) into instruction `dst`."""
    if extra is None:
        return
    si = dst.sync_info
    if si is None:
        dst.sync_info = mybir.SyncInfo(on_wait=list(extra.on_wait),
                                       on_update=list(extra.on_update))
        return
    dst.sync_info = mybir.SyncInfo(
        on_wait=list(si.on_wait) + list(extra.on_wait),
        on_update=list(si.on_update) + list(extra.on_update),
    )


def _dedup_ldweights(nc):
    """Remove InstLdweights whose stationary operand is already loaded.

    After tile scheduling the PE stream is L M L M ... with one
    legalizer-emitted LDWEIGHTS per matmul. Matmuls within a group share
    the stationary operand, and the PE keeps its weight registers
    between matmuls, so the repeats are pure overhead. Waits/updates of
    a dropped L are merged into the matmul it preceded.
    """
    n_drop = 0
    pe = mybir.EngineType.PE
    for f in nc.m.functions:
        for bb in f.blocks:
            insts = list(bb.instructions)
            keep = []
            last_sig = None
            pending = []  # candidate-dropped L's awaiting their matmul
            for inst in insts:
                if isinstance(inst, mybir.InstLdweights):
                    sig = (str(inst.ins[0]), str(inst.perf_mode),
                           str(inst.is_transpose), str(inst.tile_position))
                    if sig == last_sig:
                        pending.append(inst)
                    else:
                        # conservatively keep any unmerged pending L's
                        keep.extend(pending)
                        pending = []
                        last_sig = sig
                        keep.append(inst)
                elif isinstance(inst, mybir.InstMatmult):
                    for l in pending:
                        _merge_sync(inst, l.sync_info)
                        n_drop += 1
                    pending = []
                    keep.append(inst)
                else:
                    if getattr(inst, "engine", None) == pe:
                        # unknown PE instruction: weight state unknown
                        keep.extend(pending)
                        pending = []
                        last_sig = None
                    keep.append(inst)
            keep.extend(pending)
            if len(keep) != len(insts):
                il = bb.instructions
                try:
                    il[:] = keep
                except TypeError:
                    bb.instructions = keep
    return n_drop


_NC_CACHE = {}


def _get_nc():
    if "nc" not in _NC_CACHE:
        nc = bacc.Bacc("TRN2", target_bir_lowering=False, debug=False,
                       num_devices=N_CORES)
        xp = nc.dram_tensor("xp", [NPAIR, P, KP * 2 * MI * P], FP8,
                            kind="ExternalInput")
        wp = nc.dram_tensor("wp", [KP, P, NB * 2 * NB_W], FP8,
                            kind="ExternalInput")
        out = nc.dram_tensor("out", [M_LOCAL, N_RULES], FP8,
                             kind="ExternalOutput")
        with tile.TileContext(nc) as tc:
            _body(tc, out.ap(), xp.ap(), wp.ap())
        if DEDUP_LDW:
            n = _dedup_ldweights(nc)
            if os.environ.get("KBENCH_DEBUG"):
                print(f"[kernel] deduped {n} InstLdweights")
        nc.compile()
        _NC_CACHE["nc"] = nc
    return _NC_CACHE["nc"]


def _np_fp8():
    import ml_dtypes
    return ml_dtypes.float8_e4m3


def _rtp20(a: np.ndarray) -> np.ndarray:
    """Round positive f32 values toward +inf at fp8e4m3 mantissa
    granularity (3 bits => chop f32 mantissa at bit 20, rounding up)."""
    v = np.ascontiguousarray(a, dtype=np.float32).view(np.uint32)
    frac = v & np.uint32(0x000FFFFF)
    t = (v & ~np.uint32(0x000FFFFF)) + np.where(
        frac != 0, np.uint32(0x00100000), np.uint32(0))
    return t.view(np.float32)


def _enc_x(x_shard: np.ndarray) -> np.ndarray:
    """[M_LOCAL, IN_DIM] f32 -> [NPAIR, P, 8192] fp8 of rtp8(1 - x).

    Round-up keeps every positive (1-x) positive; exact 0 stays 0, so
    the device-side predicate sum(xe*wb) > 0 matches (1-x>0 AND W>.5).
    Values below 2^-6 are promoted to 2^-6 (still positive, still
    monotone) so no fp8 subnormals reach the PE.
    """
    t = 1.0 - np.ascontiguousarray(x_shard, dtype=np.float32)
    e = np.where(t > 0,
                 np.maximum(_rtp20(np.minimum(t, np.float32(1.0))),
                            np.float32(2.0 ** -6)),
                 np.float32(0.0)).astype(np.float32)
    e8 = e.astype(_np_fp8())
    # [m, k] -> [pair, p, (kk j mi mcol)] with
    # k = kk*256 + j*128 + p,  m = pair*256 + mi*128 + mcol
    a = e8.T.reshape(KP, 2, P, NPAIR, MI, P)   # [kk, j, p, pair, mi, mcol]
    a = a.transpose(3, 2, 0, 1, 4, 5)          # [pair, p, kk, j, mi, mcol]
    return np.ascontiguousarray(a.reshape(NPAIR, P, KP * 2 * MI * P))


def _enc_w(W: np.ndarray) -> np.ndarray:
    """[IN_DIM, N_RULES] f32 -> [KP, P, 4096] fp8 rtp (0.5 exact, so
    rtp8(W) > 0.5 <=> W > 0.5; thresholding happens on device)."""
    v = np.minimum(np.ascontiguousarray(W, dtype=np.float32),
                   np.float32(1.0))
    w8 = _rtp20(v).astype(_np_fp8())
    a = w8.reshape(KP, 2, P, NB, NB_W)   # [kk, j, p, nb, n]
    a = a.transpose(0, 2, 3, 1, 4)       # [kk, p, nb, j, n]
    return np.ascontiguousarray(a.reshape(KP, P, NB * 2 * NB_W))


def kernel(x: np.ndarray, W: np.ndarray, **run_kwargs) -> np.ndarray:
    assert x.shape == (BATCH, IN_DIM) and W.shape == (IN_DIM, N_RULES)
    nc = _get_nc()
    wp = _enc_w(W)
    in_maps = []
    for c in range(N_CORES):
        in_maps.append({"xp": _enc_x(x[c * M_LOCAL:(c + 1) * M_LOCAL, :]),
                        "wp": wp})
    res = run_bass_kernel_spmd(nc, in_maps, core_ids=list(range(N_CORES)),
                               **run_kwargs)
    out = np.concatenate([res.results[c]["out"] for c in range(N_CORES)],
                         axis=0).astype(np.float32)  # fp8 {0,1} -> f32 exact
    if run_kwargs:
        kernel.last_results = res
    return out
